# revision 46
# baseline (speedup 1.0000x reference)
"""Distributed masked-attention kernel for one TRN2 chip (8 NeuronCores).

Problem: B=4, S=4096, IN=512, D=64 attention with a [S,S] int32 score mask
(masked scores replaced by 1e-6 *before* softmax, so masked probs are
exp(1e-6)/Z ~= 1/Z, NOT zero).

Sharding (8 cores):
  core c = bg*4 + sq,  bg in {0,1} -> batches [2bg, 2bg+1],
  sq in {0..3} -> query rows [1024*sq, 1024*(sq+1)).
  Per-core inputs (layout chosen at scatter time):
    xt    = embedding[2bg:2bg+2].transpose(0,2,1)   [2, 512, 4096] f32
    maskt = mask[q_slab, :].T                       [4096, 1024]  int32
  Both are rolled along S so the core's own query slab is at rows [0:1024)
  (attention's k-sum is permutation invariant), letting all 8 cores run the
  IDENTICAL graph (SPMD).

Per-core device pipeline:
  QKV (per 1024-wide S-chunk, both batches interleaved so the DMA queue mixes
  x chunks with mask tiles): xt chunk cast f32->bf16 on DVE; K^T/V^T via
  packed [Wk|Wv] bf16 matmuls (biases fused into the ACT PSUM->SBUF copy);
  V^T chunk PE-transposed into V_aug=[V|1] tiles (ones column => the PV
  matmul emits the softmax denominator for free).
  Attention in the transposed domain S^T[k,q], both batches per k-tile so
  each streamed int32 mask tile is consumed twice and never stored:
    PE:  S^T = (K^T block)^T @ Q^T            (bf16, 2x N=512; Q^T is
         zero-padded to 128 partitions so the contraction uses the full PE
         array - required to un-throttle the PE HAM clock gate to 2.4GHz)
    DVE: sm = S^T * mask  (in-place PSUM)     (tensor_tensor, PSUM x int32)
    ACT: P = exp(0.125 * sm)                  (masked -> exp(0) = 1, matching
                                               the reference's exp(1e-6))
    PE:  O^T[65, q] += V_aug^T @ P            (2x N=512)
  The PV matmuls are emitted lagging one k-tile behind the scores so the
  in-order PE queue never head-of-line blocks on the DVE/ACT chain.
  Deep staging pools (xs x10, mask x16) keep >=10 DMA transfers in flight:
  each HWDGE dma_start runs on ONE queue at ~27GB/s, so aggregate HBM
  bandwidth equals 27GB/s x outstanding transfers.
  Epilogue: PE-transpose O^T, divide by the denominator row, DMA out.
"""

import sys

if "/opt/trn_rl_repo" not in sys.path:
    sys.path.insert(0, "/opt/trn_rl_repo")

from contextlib import ExitStack

import numpy as np

import concourse.bass as bass
import concourse.bacc as bacc
import concourse.mybir as mybir
import concourse.tile as tile
from concourse.bass_utils import run_bass_kernel_spmd
from concourse.masks import make_identity

ts = bass.ts
ds = bass.ds

N_CORES = 8
B, S, C, D = 4, 4096, 512, 64
B_LOC = 2          # batches per core
Q_LOC = 1024       # query rows per core
N_KT = S // 128    # 32 k-tiles of 128
QC = 512           # matmul moving chunk

F32 = mybir.dt.float32
F32R = mybir.dt.float32r
BF16 = mybir.dt.bfloat16
I32 = mybir.dt.int32
AF = mybir.ActivationFunctionType
ALU = mybir.AluOpType


def build_kernel() -> bacc.Bacc:
    nc = bacc.Bacc(None, target_bir_lowering=False, debug=False)

    xt_ext = nc.declare_dram_parameter("xt", [B_LOC, C, S], F32, isOutput=False)
    mt_ext = nc.declare_dram_parameter("maskt", [S, Q_LOC], I32, isOutput=False)
    wq_ext = nc.declare_dram_parameter("wq", [C, D], F32, isOutput=False)
    bq_ext = nc.declare_dram_parameter("bq", [D], F32, isOutput=False)
    wk_ext = nc.declare_dram_parameter("wk", [C, D], F32, isOutput=False)
    bk_ext = nc.declare_dram_parameter("bk", [D], F32, isOutput=False)
    wv_ext = nc.declare_dram_parameter("wv", [C, D], F32, isOutput=False)
    bv_ext = nc.declare_dram_parameter("bv", [D], F32, isOutput=False)
    out_ext = nc.declare_dram_parameter("out", [B_LOC, Q_LOC, D], F32, isOutput=True)

    with tile.TileContext(nc) as tc, ExitStack() as ctx:
        # ---------------- pools ----------------
        persist = ctx.enter_context(tc.tile_pool(name="persist", bufs=1))
        xt_pool = ctx.enter_context(tc.tile_pool(name="xtp", bufs=3))
        mstage = ctx.enter_context(tc.tile_pool(name="mstage", bufs=16))
        xstage = ctx.enter_context(tc.tile_pool(name="xstage", bufs=10))
        pt_pool = ctx.enter_context(tc.tile_pool(name="pt", bufs=6))
        epi = ctx.enter_context(tc.tile_pool(name="epi", bufs=1))
        epi2 = ctx.enter_context(tc.tile_pool(name="epi2", bufs=2))
        psum_s = ctx.enter_context(
            tc.tile_pool(name="psum_s", bufs=2, space=bass.MemorySpace.PSUM)
        )
        psum_o = ctx.enter_context(
            tc.tile_pool(name="psum_o", bufs=2, space=bass.MemorySpace.PSUM)
        )

        # ---------------- constants / weights ----------------
        ident_f = persist.tile([128, 128], F32)
        make_identity(nc, ident_f[:])
        ident_b = persist.tile([128, 128], BF16)
        make_identity(nc, ident_b[:])
        ones_col = persist.tile([128, 1], BF16)
        nc.gpsimd.memset(ones_col[:], 1.0)

        # [Wk | Wv] packed bf16 stationary blocks; Wq separate
        w_f32 = persist.tile([128, 4, 2 * D], F32)
        wq_f32 = persist.tile([128, 4, D], F32)
        nc.sync.dma_start(w_f32[:, :, 0:D], wk_ext[:].rearrange("(j p) d -> p j d", p=128))
        nc.sync.dma_start(w_f32[:, :, D : 2 * D], wv_ext[:].rearrange("(j p) d -> p j d", p=128))
        nc.sync.dma_start(wq_f32[:], wq_ext[:].rearrange("(j p) d -> p j d", p=128))
        wkv = persist.tile([128, 4, 2 * D], BF16)
        wq = persist.tile([128, 4, D], BF16)
        nc.vector.tensor_copy(wkv[:], w_f32[:])
        nc.vector.tensor_copy(wq[:], wq_f32[:])

        bias_kv = persist.tile([128, 1], F32)
        nc.sync.dma_start(bias_kv[0:D, :], bk_ext[:].rearrange("(a b) -> a b", b=1))
        nc.sync.dma_start(bias_kv[D : 2 * D, :], bv_ext[:].rearrange("(a b) -> a b", b=1))
        bias_q = persist.tile([D, 1], F32)
        nc.sync.dma_start(bias_q[:], bq_ext[:].rearrange("(a b) -> a b", b=1))

        # ---------------- persistent per-batch tensors ----------------
        kvt = [persist.tile([128, S], BF16, name=f"kvt{b}", tag=f"kvt{b}") for b in range(B_LOC)]
        xtc = {}  # (b, c) -> rotating chunk tile [128, 4, 1024]
        qt_t = [persist.tile([128, Q_LOC], BF16, name=f"qt{b}", tag=f"qt{b}") for b in range(B_LOC)]
        vaug = [persist.tile([128, N_KT, D + 1], BF16, name=f"va{b}", tag=f"va{b}") for b in range(B_LOC)]

        def load_mask(kt, pool, tag):
            mk = pool.tile([128, Q_LOC], I32, name=f"mk{kt}", tag=tag)
            nc.sync.dma_start(mk[:], mt_ext[ts(kt, 128), :])
            return mk

        def emit_x_load(b: int, c: int, j: int):
            if (b, c) not in xtc:
                xtc[(b, c)] = xt_pool.tile(
                    [128, 4, 1024], BF16, name=f"xtc{b}_{c}", tag="xtc"
                )
            xs = xstage.tile([128, 1024], F32, tag="xs")
            if c == 0:
                for v in range(2):
                    nc.sync.dma_start(
                        xs[:, ts(v, QC)],
                        xt_ext[b, ts(j, 128), ds(c * 1024 + v * QC, QC)],
                    )
            else:
                nc.sync.dma_start(xs[:], xt_ext[b, ts(j, 128), ts(c, 1024)])
            if j % 2 == 0:
                nc.vector.tensor_copy(xtc[(b, c)][:, j, :], xs[:])
            else:
                nc.scalar.copy(xtc[(b, c)][:, j, :], xs[:])

        def emit_kv_half(b: int, c: int, h: int):
            # K^T|V^T for a 512-wide piece; one ps slot for ~2.5us only
            kv_ps = psum_s.tile([128, QC], F32, name="kvps", tag="ps")
            for j in range(4):
                nc.tensor.matmul(
                    kv_ps[:],
                    wkv[:, j, :],
                    xtc[(b, c)][:, j, ts(h, QC)],
                    start=(j == 0),
                    stop=(j == 3),
                )
            nc.scalar.activation(
                kvt[b][:, ds(c * 1024 + h * QC, QC)], kv_ps[:], AF.Identity,
                bias=bias_kv[:],
            )
            # V_aug rows for this half's 4 k-tiles via PE transpose
            vp = psum_s.tile([128, 4, D], BF16, name="vp", tag="ps")
            kt0 = 8 * c + 4 * h
            for u in range(4):
                nc.tensor.transpose(
                    vp[:, u, :],
                    kvt[b][D : 2 * D, ts(kt0 + u, 128)],
                    ident_b[D : 2 * D, D : 2 * D],
                )
            nc.scalar.copy(vaug[b][:, kt0 : kt0 + 4, 0:D], vp[:])

        def emit_q(b: int):
            q_ps = psum_s.tile([D, Q_LOC], F32, name="qps", tag="ps")
            for h in range(Q_LOC // QC):
                for j in range(4):
                    nc.tensor.matmul(
                        q_ps[:, ts(h, QC)],
                        wq[:, j, :],
                        xtc[(b, 0)][:, j, ts(h, QC)],
                        start=(j == 0),
                        stop=(j == 3),
                    )
            nc.scalar.activation(qt_t[b][0:D, :], q_ps[:], AF.Identity, bias=bias_q[:])
            nc.gpsimd.memset(qt_t[b][D:128, :], 0.0)
            nc.gpsimd.memset(vaug[b][:, :, D : D + 1], 1.0)

        def emit_scores_exp(b, kt, mk):
            st = psum_s.tile([128, Q_LOC], F32, name="st", tag="ps")
            for qc in range(Q_LOC // QC):
                nc.tensor.matmul(
                    st[:, ts(qc, QC)],
                    kvt[b][:, ts(kt, 128)],
                    qt_t[b][:, ts(qc, QC)],
                    start=True,
                    stop=True,
                )
            nc.vector.tensor_tensor(out=st[:], in0=st[:], in1=mk[:], op=ALU.mult)
            pt = pt_pool.tile([128, Q_LOC], BF16, tag="pt")
            nc.scalar.activation(pt[:], st[:], AF.Exp, scale=0.125)
            return pt

        def emit_pv(b, kt, ot, pt, first, last):
            for qc in range(Q_LOC // QC):
                nc.tensor.matmul(
                    ot[:, ts(qc, QC)],
                    vaug[b][:, kt, :],
                    pt[:, ts(qc, QC)],
                    start=first,
                    stop=last,
                )

        def emit_epilogue(b, ot):
            ots = epi.tile([D + 1, Q_LOC], F32, tag="ots")
            nc.scalar.copy(ots[:], ot[:])
            for qt in range(Q_LOC // 128):
                op = psum_s.tile([128, D + 1], F32, name="op", tag="ps")
                nc.tensor.transpose(
                    op[:], ots[:, ts(qt, 128)], ident_f[0 : D + 1, 0 : D + 1]
                )
                rcp = epi2.tile([128, 1], F32, tag="rcp")
                nc.vector.reciprocal(rcp[:], op[:, D : D + 1])
                of = epi2.tile([128, D], F32, tag="of")
                nc.vector.tensor_scalar(of[:], op[:, 0:D], rcp[:], None, op0=ALU.mult)
                nc.sync.dma_start(out_ext[b, ts(qt, 128), :], of[:])

        # ---------------- emission order (overlap hint) ----------------
        # PV matmuls lag one k-tile behind the scores so the in-order PE queue
        # never head-of-line blocks on the DVE/ACT chain (keeps PE dense/warm).
        ot0 = psum_o.tile([D + 1, Q_LOC], F32, name="ot0", tag="ot")
        ot1 = psum_o.tile([D + 1, Q_LOC], F32, name="ot1", tag="ot")
        ots = [ot0, ot1]
        N_C = 4
        for b in range(B_LOC):
            for j in range(4):
                emit_x_load(b, 0, j)
        for b in range(B_LOC):
            emit_kv_half(b, 0, 0)
            emit_kv_half(b, 0, 1)
            emit_q(b)
        pending = None  # (kt, [pt_b0, pt_b1])
        for c in range(N_C):
            nxt = []
            if c + 1 < N_C:
                nxt = (
                    [("x", b, c + 1, j) for b in range(B_LOC) for j in range(4)]
                    + [("kv", b, c + 1, h) for b in range(B_LOC) for h in range(2)]
                )
            for i, kt in enumerate(range(8 * c, 8 * c + 8)):
                mk = load_mask(kt, mstage, "mk")
                pts = [emit_scores_exp(b, kt, mk) for b in range(B_LOC)]
                if pending is not None:
                    pkt, ppts = pending
                    for b in range(B_LOC):
                        emit_pv(b, pkt, ots[b], ppts[b], pkt == 0, False)
                pending = (kt, pts)
                take = 2 if i < 4 else 1
                for _ in range(min(take, len(nxt))):
                    piece = nxt.pop(0)
                    if piece[0] == "x":
                        emit_x_load(*piece[1:])
                    else:
                        emit_kv_half(*piece[1:])
            for piece in nxt:
                if piece[0] == "x":
                    emit_x_load(*piece[1:])
                else:
                    emit_kv_half(*piece[1:])
        pkt, ppts = pending
        for b in range(B_LOC):
            emit_pv(b, pkt, ots[b], ppts[b], False, True)
        emit_epilogue(0, ot0)
        emit_epilogue(1, ot1)

    nc.compile()
    return nc


def _shard_inputs(input_embedding, mask, Wq, bq, Wk, bk, Wv, bv):
    input_embedding = np.asarray(input_embedding, dtype=np.float32)
    mask = np.asarray(mask, dtype=np.int32)
    w = {
        "wq": np.ascontiguousarray(np.asarray(Wq, np.float32)),
        "bq": np.ascontiguousarray(np.asarray(bq, np.float32)),
        "wk": np.ascontiguousarray(np.asarray(Wk, np.float32)),
        "bk": np.ascontiguousarray(np.asarray(bk, np.float32)),
        "wv": np.ascontiguousarray(np.asarray(Wv, np.float32)),
        "bv": np.ascontiguousarray(np.asarray(bv, np.float32)),
    }
    in_maps = []
    for c in range(N_CORES):
        bg, sq = divmod(c, 4)
        # x^T layout [2, C, S]; roll S so this core's q-slab is at [0:Q_LOC)
        x_c = np.roll(
            input_embedding[2 * bg : 2 * bg + 2].transpose(0, 2, 1),
            -Q_LOC * sq,
            axis=2,
        )
        # mask^T slab [S(k), Q_LOC(q)]; roll k-axis identically
        m_c = np.roll(mask[Q_LOC * sq : Q_LOC * (sq + 1), :].T, -Q_LOC * sq, axis=0)
        in_maps.append(
            {
                "xt": np.ascontiguousarray(x_c),
                "maskt": np.ascontiguousarray(m_c),
                **w,
            }
        )
    return in_maps


def _gather(results):
    out = np.empty((B, S, D), dtype=np.float32)
    for c in range(N_CORES):
        bg, sq = divmod(c, 4)
        out[2 * bg : 2 * bg + 2, Q_LOC * sq : Q_LOC * (sq + 1), :] = results[c]["out"]
    return out


def kernel(input_embedding, mask, Wq, bq, Wk, bk, Wv, bv):
    nc = build_kernel()
    in_maps = _shard_inputs(input_embedding, mask, Wq, bq, Wk, bk, Wv, bv)
    res = run_bass_kernel_spmd(nc, in_maps, list(range(N_CORES)))
    return _gather(res.results)


# revision 47
# speedup vs baseline: 1.1829x; 1.1829x over previous
"""Distributed masked-attention kernel for one TRN2 chip (8 NeuronCores).

Problem: B=4, S=4096, IN=512, D=64 attention with a [S,S] int32 score mask
(masked scores replaced by 1e-6 *before* softmax, so masked probs are
exp(1e-6)/Z ~= 1/Z, NOT zero).

Sharding (8 cores):
  core c = bg*4 + sq,  bg in {0,1} -> batches [2bg, 2bg+1],
  sq in {0..3} -> query rows [1024*sq, 1024*(sq+1)).
  Per-core inputs (layout chosen at scatter time):
    xt    = embedding[2bg:2bg+2].transpose(0,2,1)   [2, 512, 4096] f32
    maskt = mask[q_slab, :].T                       [4096, 1024]  int32
  Both are rolled along S so the core's own query slab is at rows [0:1024)
  (attention's k-sum is permutation invariant), letting all 8 cores run the
  IDENTICAL graph (SPMD).

Per-core device pipeline:
  QKV (per 1024-wide S-chunk, both batches interleaved so the DMA queue mixes
  x chunks with mask tiles): xt chunk cast f32->bf16 on DVE; K^T/V^T via
  packed [Wk|Wv] bf16 matmuls (biases fused into the ACT PSUM->SBUF copy);
  V^T chunk PE-transposed into V_aug=[V|1] tiles (ones column => the PV
  matmul emits the softmax denominator for free).
  Attention in the transposed domain S^T[k,q], both batches per k-tile so
  each streamed int32 mask tile is consumed twice and never stored:
    PE:  S^T = (K^T block)^T @ Q^T            (bf16, 2x N=512; Q^T is
         zero-padded to 128 partitions so the contraction uses the full PE
         array - required to un-throttle the PE HAM clock gate to 2.4GHz)
    DVE: sm = S^T * mask  (in-place PSUM)     (tensor_tensor, PSUM x int32)
    ACT: P = exp(0.125 * sm)                  (masked -> exp(0) = 1, matching
                                               the reference's exp(1e-6))
    PE:  O^T[65, q] += V_aug^T @ P            (2x N=512)
  The PV matmuls are emitted lagging one k-tile behind the scores so the
  in-order PE queue never head-of-line blocks on the DVE/ACT chain.
  Deep staging pools (xs x10, mask x16) keep >=10 DMA transfers in flight:
  each HWDGE dma_start runs on ONE queue at ~27GB/s, so aggregate HBM
  bandwidth equals 27GB/s x outstanding transfers.
  Epilogue: PE-transpose O^T, divide by the denominator row, DMA out.
"""

import sys

if "/opt/trn_rl_repo" not in sys.path:
    sys.path.insert(0, "/opt/trn_rl_repo")

from contextlib import ExitStack

import numpy as np

import concourse.bass as bass
import concourse.bacc as bacc
import concourse.mybir as mybir
import concourse.tile as tile
from concourse.bass_utils import run_bass_kernel_spmd
from concourse.masks import make_identity

ts = bass.ts
ds = bass.ds

N_CORES = 8
B, S, C, D = 4, 4096, 512, 64
B_LOC = 2          # batches per core
Q_LOC = 1024       # query rows per core
N_KT = S // 128    # 32 k-tiles of 128
QC = 512           # matmul moving chunk

F32 = mybir.dt.float32
F32R = mybir.dt.float32r
BF16 = mybir.dt.bfloat16
I32 = mybir.dt.int32
AF = mybir.ActivationFunctionType
ALU = mybir.AluOpType


def build_kernel() -> bacc.Bacc:
    nc = bacc.Bacc(None, target_bir_lowering=False, debug=False)

    xt_ext = nc.declare_dram_parameter("xt", [B_LOC, C, S], F32, isOutput=False)
    mt_ext = nc.declare_dram_parameter("maskt", [S, Q_LOC], I32, isOutput=False)
    wq_ext = nc.declare_dram_parameter("wq", [C, D], F32, isOutput=False)
    bq_ext = nc.declare_dram_parameter("bq", [D], F32, isOutput=False)
    wk_ext = nc.declare_dram_parameter("wk", [C, D], F32, isOutput=False)
    bk_ext = nc.declare_dram_parameter("bk", [D], F32, isOutput=False)
    wv_ext = nc.declare_dram_parameter("wv", [C, D], F32, isOutput=False)
    bv_ext = nc.declare_dram_parameter("bv", [D], F32, isOutput=False)
    out_ext = nc.declare_dram_parameter("out", [B_LOC, Q_LOC, D], F32, isOutput=True)

    with tile.TileContext(nc) as tc, ExitStack() as ctx:
        # ---------------- pools ----------------
        persist = ctx.enter_context(tc.tile_pool(name="persist", bufs=1))
        xt_pool = ctx.enter_context(tc.tile_pool(name="xtp", bufs=3))
        mstage = ctx.enter_context(tc.tile_pool(name="mstage", bufs=16))
        xstage = ctx.enter_context(tc.tile_pool(name="xstage", bufs=10))
        pt_pool = ctx.enter_context(tc.tile_pool(name="pt", bufs=6))
        epi = ctx.enter_context(tc.tile_pool(name="epi", bufs=1))
        epi2 = ctx.enter_context(tc.tile_pool(name="epi2", bufs=2))
        psum_s = ctx.enter_context(
            tc.tile_pool(name="psum_s", bufs=2, space=bass.MemorySpace.PSUM)
        )
        psum_o = ctx.enter_context(
            tc.tile_pool(name="psum_o", bufs=2, space=bass.MemorySpace.PSUM)
        )

        # ---------------- constants / weights ----------------
        ident_f = persist.tile([128, 128], F32)
        make_identity(nc, ident_f[:])
        ident_b = persist.tile([128, 128], BF16)
        make_identity(nc, ident_b[:])
        ones_col = persist.tile([128, 1], BF16)
        nc.gpsimd.memset(ones_col[:], 1.0)

        # [Wk | Wv] packed bf16 stationary blocks; Wq separate
        w_f32 = persist.tile([128, 4, 2 * D], F32)
        wq_f32 = persist.tile([128, 4, D], F32)
        nc.sync.dma_start(w_f32[:, :, 0:D], wk_ext[:].rearrange("(j p) d -> p j d", p=128))
        nc.sync.dma_start(w_f32[:, :, D : 2 * D], wv_ext[:].rearrange("(j p) d -> p j d", p=128))
        nc.sync.dma_start(wq_f32[:], wq_ext[:].rearrange("(j p) d -> p j d", p=128))
        wkv = persist.tile([128, 4, 2 * D], BF16)
        wq = persist.tile([128, 4, D], BF16)
        nc.vector.tensor_copy(wkv[:], w_f32[:])
        nc.vector.tensor_copy(wq[:], wq_f32[:])

        bias_kv = persist.tile([128, 1], F32)
        nc.sync.dma_start(bias_kv[0:D, :], bk_ext[:].rearrange("(a b) -> a b", b=1))
        nc.sync.dma_start(bias_kv[D : 2 * D, :], bv_ext[:].rearrange("(a b) -> a b", b=1))
        bias_q = persist.tile([D, 1], F32)
        nc.sync.dma_start(bias_q[:], bq_ext[:].rearrange("(a b) -> a b", b=1))

        # ---------------- persistent per-batch tensors ----------------
        kvt = [persist.tile([128, S], BF16, name=f"kvt{b}", tag=f"kvt{b}") for b in range(B_LOC)]
        xtc = {}  # (b, c) -> rotating chunk tile [128, 4, 1024]
        qt_t = [persist.tile([128, Q_LOC], BF16, name=f"qt{b}", tag=f"qt{b}") for b in range(B_LOC)]
        vaug = [persist.tile([128, N_KT, D + 1], BF16, name=f"va{b}", tag=f"va{b}") for b in range(B_LOC)]

        def load_mask(kt, pool, tag):
            mk = pool.tile([128, Q_LOC], I32, name=f"mk{kt}", tag=tag)
            nc.sync.dma_start(mk[:], mt_ext[ts(kt, 128), :])
            return mk

        def emit_x_load(b: int, c: int, j: int):
            if (b, c) not in xtc:
                xtc[(b, c)] = xt_pool.tile(
                    [128, 4, 1024], BF16, name=f"xtc{b}_{c}", tag="xtc"
                )
            xs = xstage.tile([128, 1024], F32, tag="xs")
            nc.sync.dma_start(xs[:], xt_ext[b, ts(j, 128), ts(c, 1024)])
            if j % 2 == 0:
                nc.vector.tensor_copy(xtc[(b, c)][:, j, :], xs[:])
            else:
                nc.scalar.copy(xtc[(b, c)][:, j, :], xs[:])

        def emit_kv_half(b: int, c: int, h: int):
            # K^T|V^T for a 512-wide piece; one ps slot for ~2.5us only
            kv_ps = psum_s.tile([128, QC], F32, name="kvps", tag="ps")
            for j in range(4):
                nc.tensor.matmul(
                    kv_ps[:],
                    wkv[:, j, :],
                    xtc[(b, c)][:, j, ts(h, QC)],
                    start=(j == 0),
                    stop=(j == 3),
                )
            nc.scalar.activation(
                kvt[b][:, ds(c * 1024 + h * QC, QC)], kv_ps[:], AF.Identity,
                bias=bias_kv[:],
            )
            # V_aug rows for this half's 4 k-tiles via PE transpose
            vp = psum_s.tile([128, 4, D], BF16, name="vp", tag="ps")
            kt0 = 8 * c + 4 * h
            for u in range(4):
                nc.tensor.transpose(
                    vp[:, u, :],
                    kvt[b][D : 2 * D, ts(kt0 + u, 128)],
                    ident_b[D : 2 * D, D : 2 * D],
                )
            nc.scalar.copy(vaug[b][:, kt0 : kt0 + 4, 0:D], vp[:])

        def emit_q(b: int):
            q_ps = psum_s.tile([D, Q_LOC], F32, name="qps", tag="ps")
            for h in range(Q_LOC // QC):
                for j in range(4):
                    nc.tensor.matmul(
                        q_ps[:, ts(h, QC)],
                        wq[:, j, :],
                        xtc[(b, 0)][:, j, ts(h, QC)],
                        start=(j == 0),
                        stop=(j == 3),
                    )
            nc.scalar.activation(qt_t[b][0:D, :], q_ps[:], AF.Identity, bias=bias_q[:])
            nc.gpsimd.memset(qt_t[b][D:128, :], 0.0)
            nc.gpsimd.memset(vaug[b][:, :, D : D + 1], 1.0)

        def emit_scores_exp(b, kt, mk):
            st = psum_s.tile([128, Q_LOC], F32, name="st", tag="ps")
            for qc in range(Q_LOC // QC):
                nc.tensor.matmul(
                    st[:, ts(qc, QC)],
                    kvt[b][:, ts(kt, 128)],
                    qt_t[b][:, ts(qc, QC)],
                    start=True,
                    stop=True,
                )
            nc.vector.tensor_tensor(out=st[:], in0=st[:], in1=mk[:], op=ALU.mult)
            pt = pt_pool.tile([128, Q_LOC], BF16, tag="pt")
            nc.scalar.activation(pt[:], st[:], AF.Exp, scale=0.125)
            return pt

        def emit_pv(b, kt, ot, pt, first, last):
            for qc in range(Q_LOC // QC):
                nc.tensor.matmul(
                    ot[:, ts(qc, QC)],
                    vaug[b][:, kt, :],
                    pt[:, ts(qc, QC)],
                    start=first,
                    stop=last,
                )

        def emit_epilogue(b, ot):
            ots = epi.tile([D + 1, Q_LOC], F32, tag="ots")
            nc.scalar.copy(ots[:], ot[:])
            for qt in range(Q_LOC // 128):
                op = psum_s.tile([128, D + 1], F32, name="op", tag="ps")
                nc.tensor.transpose(
                    op[:], ots[:, ts(qt, 128)], ident_f[0 : D + 1, 0 : D + 1]
                )
                rcp = epi2.tile([128, 1], F32, tag="rcp")
                nc.vector.reciprocal(rcp[:], op[:, D : D + 1])
                of = epi2.tile([128, D], F32, tag="of")
                nc.vector.tensor_scalar(of[:], op[:, 0:D], rcp[:], None, op0=ALU.mult)
                nc.sync.dma_start(out_ext[b, ts(qt, 128), :], of[:])

        # ---------------- emission order (overlap hint) ----------------
        # PV matmuls lag one k-tile behind the scores so the in-order PE queue
        # never head-of-line blocks on the DVE/ACT chain (keeps PE dense/warm).
        ot0 = psum_o.tile([D + 1, Q_LOC], F32, name="ot0", tag="ot")
        ot1 = psum_o.tile([D + 1, Q_LOC], F32, name="ot1", tag="ot")
        ots = [ot0, ot1]
        N_C = 4
        for b in range(B_LOC):
            for j in range(4):
                emit_x_load(b, 0, j)
        for b in range(B_LOC):
            emit_kv_half(b, 0, 0)
            emit_kv_half(b, 0, 1)
            emit_q(b)
        pending = []  # [(kt, [pt_b0, pt_b1]), ...]
        for c in range(N_C):
            nxt = []
            if c + 1 < N_C:
                nxt = (
                    [("x", b, c + 1, j) for b in range(B_LOC) for j in range(4)]
                    + [("kv", b, c + 1, h) for b in range(B_LOC) for h in range(2)]
                )
            for i, kt in enumerate(range(8 * c, 8 * c + 8)):
                mk = load_mask(kt, mstage, "mk")
                pts = [emit_scores_exp(b, kt, mk) for b in range(B_LOC)]
                pending.append((kt, pts))
                if len(pending) > 2:
                    pkt, ppts = pending.pop(0)
                    for b in range(B_LOC):
                        emit_pv(b, pkt, ots[b], ppts[b], pkt == 0, False)
                take = 2 if i < 4 else 1
                for _ in range(min(take, len(nxt))):
                    piece = nxt.pop(0)
                    if piece[0] == "x":
                        emit_x_load(*piece[1:])
                    else:
                        emit_kv_half(*piece[1:])
            for piece in nxt:
                if piece[0] == "x":
                    emit_x_load(*piece[1:])
                else:
                    emit_kv_half(*piece[1:])
        for idx, (pkt, ppts) in enumerate(pending):
            for b in range(B_LOC):
                emit_pv(b, pkt, ots[b], ppts[b], pkt == 0, pkt == N_KT - 1)
        emit_epilogue(0, ot0)
        emit_epilogue(1, ot1)

    nc.compile()
    return nc


def _shard_inputs(input_embedding, mask, Wq, bq, Wk, bk, Wv, bv):
    input_embedding = np.asarray(input_embedding, dtype=np.float32)
    mask = np.asarray(mask, dtype=np.int32)
    w = {
        "wq": np.ascontiguousarray(np.asarray(Wq, np.float32)),
        "bq": np.ascontiguousarray(np.asarray(bq, np.float32)),
        "wk": np.ascontiguousarray(np.asarray(Wk, np.float32)),
        "bk": np.ascontiguousarray(np.asarray(bk, np.float32)),
        "wv": np.ascontiguousarray(np.asarray(Wv, np.float32)),
        "bv": np.ascontiguousarray(np.asarray(bv, np.float32)),
    }
    in_maps = []
    for c in range(N_CORES):
        bg, sq = divmod(c, 4)
        # x^T layout [2, C, S]; roll S so this core's q-slab is at [0:Q_LOC)
        x_c = np.roll(
            input_embedding[2 * bg : 2 * bg + 2].transpose(0, 2, 1),
            -Q_LOC * sq,
            axis=2,
        )
        # mask^T slab [S(k), Q_LOC(q)]; roll k-axis identically
        m_c = np.roll(mask[Q_LOC * sq : Q_LOC * (sq + 1), :].T, -Q_LOC * sq, axis=0)
        in_maps.append(
            {
                "xt": np.ascontiguousarray(x_c),
                "maskt": np.ascontiguousarray(m_c),
                **w,
            }
        )
    return in_maps


def _gather(results):
    out = np.empty((B, S, D), dtype=np.float32)
    for c in range(N_CORES):
        bg, sq = divmod(c, 4)
        out[2 * bg : 2 * bg + 2, Q_LOC * sq : Q_LOC * (sq + 1), :] = results[c]["out"]
    return out


def kernel(input_embedding, mask, Wq, bq, Wk, bk, Wv, bv):
    nc = build_kernel()
    in_maps = _shard_inputs(input_embedding, mask, Wq, bq, Wk, bk, Wv, bv)
    res = run_bass_kernel_spmd(nc, in_maps, list(range(N_CORES)))
    return _gather(res.results)


# revision 48
# speedup vs baseline: 1.2061x; 1.0196x over previous
"""Distributed masked-attention kernel for one TRN2 chip (8 NeuronCores).

Problem: B=4, S=4096, IN=512, D=64 attention with a [S,S] int32 score mask
(masked scores replaced by 1e-6 *before* softmax, so masked probs are
exp(1e-6)/Z ~= 1/Z, NOT zero).

Sharding (8 cores):
  core c = bg*4 + sq,  bg in {0,1} -> batches [2bg, 2bg+1],
  sq in {0..3} -> query rows [1024*sq, 1024*(sq+1)).
  Per-core inputs (layout chosen at scatter time):
    xt    = embedding[2bg:2bg+2].transpose(0,2,1)   [2, 512, 4096] f32
    maskt = mask[q_slab, :].T                       [4096, 1024]  int32
  Both are rolled along S so the core's own query slab is at rows [0:1024)
  (attention's k-sum is permutation invariant), letting all 8 cores run the
  IDENTICAL graph (SPMD).

Per-core device pipeline:
  QKV (per 1024-wide S-chunk, both batches interleaved so the DMA queue mixes
  x chunks with mask tiles): xt chunk cast f32->bf16 on DVE; K^T/V^T via
  packed [Wk|Wv] bf16 matmuls (biases fused into the ACT PSUM->SBUF copy);
  V^T chunk PE-transposed into V_aug=[V|1] tiles (ones column => the PV
  matmul emits the softmax denominator for free).
  Attention in the transposed domain S^T[k,q], both batches per k-tile so
  each streamed int32 mask tile is consumed twice and never stored:
    PE:  S^T = (K^T block)^T @ Q^T            (bf16, 2x N=512; Q^T is
         zero-padded to 128 partitions so the contraction uses the full PE
         array - required to un-throttle the PE HAM clock gate to 2.4GHz)
    DVE: sm = S^T * mask  (in-place PSUM)     (tensor_tensor, PSUM x int32)
    ACT: P = exp(0.125 * sm)                  (masked -> exp(0) = 1, matching
                                               the reference's exp(1e-6))
    PE:  O^T[65, q] += V_aug^T @ P            (2x N=512)
  The PV matmuls are emitted lagging one k-tile behind the scores so the
  in-order PE queue never head-of-line blocks on the DVE/ACT chain.
  Deep staging pools (xs x10, mask x16) keep >=10 DMA transfers in flight:
  each HWDGE dma_start runs on ONE queue at ~27GB/s, so aggregate HBM
  bandwidth equals 27GB/s x outstanding transfers.
  Epilogue: PE-transpose O^T, divide by the denominator row, DMA out.
"""

import sys

if "/opt/trn_rl_repo" not in sys.path:
    sys.path.insert(0, "/opt/trn_rl_repo")

from contextlib import ExitStack

import numpy as np

import concourse.bass as bass
import concourse.bacc as bacc
import concourse.mybir as mybir
import concourse.tile as tile
from concourse.bass_utils import run_bass_kernel_spmd
from concourse.masks import make_identity

ts = bass.ts
ds = bass.ds

N_CORES = 8
B, S, C, D = 4, 4096, 512, 64
B_LOC = 2          # batches per core
Q_LOC = 1024       # query rows per core
N_KT = S // 128    # 32 k-tiles of 128
QC = 512           # matmul moving chunk

F32 = mybir.dt.float32
F32R = mybir.dt.float32r
BF16 = mybir.dt.bfloat16
I32 = mybir.dt.int32
AF = mybir.ActivationFunctionType
ALU = mybir.AluOpType


def build_kernel() -> bacc.Bacc:
    nc = bacc.Bacc(None, target_bir_lowering=False, debug=False)

    xt_ext = nc.declare_dram_parameter("xt", [B_LOC, C, S], F32, isOutput=False)
    mt_ext = nc.declare_dram_parameter("maskt", [S, Q_LOC], I32, isOutput=False)
    wq_ext = nc.declare_dram_parameter("wq", [C, D], F32, isOutput=False)
    bq_ext = nc.declare_dram_parameter("bq", [D], F32, isOutput=False)
    wk_ext = nc.declare_dram_parameter("wk", [C, D], F32, isOutput=False)
    bk_ext = nc.declare_dram_parameter("bk", [D], F32, isOutput=False)
    wv_ext = nc.declare_dram_parameter("wv", [C, D], F32, isOutput=False)
    bv_ext = nc.declare_dram_parameter("bv", [D], F32, isOutput=False)
    out_ext = nc.declare_dram_parameter("out", [B_LOC, Q_LOC, D], F32, isOutput=True)

    with tile.TileContext(nc) as tc, ExitStack() as ctx:
        # ---------------- pools ----------------
        persist = ctx.enter_context(tc.tile_pool(name="persist", bufs=1))
        xt_pool = ctx.enter_context(tc.tile_pool(name="xtp", bufs=3))
        mstage = ctx.enter_context(tc.tile_pool(name="mstage", bufs=16))
        xstage = ctx.enter_context(tc.tile_pool(name="xstage", bufs=10))
        pt_pool = ctx.enter_context(tc.tile_pool(name="pt", bufs=6))
        epi = ctx.enter_context(tc.tile_pool(name="epi", bufs=1))
        epi2 = ctx.enter_context(tc.tile_pool(name="epi2", bufs=2))
        psum_s = ctx.enter_context(
            tc.tile_pool(name="psum_s", bufs=2, space=bass.MemorySpace.PSUM)
        )
        psum_o = ctx.enter_context(
            tc.tile_pool(name="psum_o", bufs=2, space=bass.MemorySpace.PSUM)
        )

        # ---------------- constants / weights ----------------
        ident_f = persist.tile([128, 128], F32)
        make_identity(nc, ident_f[:])
        ident_b = persist.tile([128, 128], BF16)
        make_identity(nc, ident_b[:])
        ones_col = persist.tile([128, 1], BF16)
        nc.gpsimd.memset(ones_col[:], 1.0)

        # [Wk | Wv] packed bf16 stationary blocks; Wq separate
        w_f32 = persist.tile([128, 4, 2 * D], F32)
        wq_f32 = persist.tile([128, 4, D], F32)
        nc.sync.dma_start(w_f32[:, :, 0:D], wk_ext[:].rearrange("(j p) d -> p j d", p=128))
        nc.sync.dma_start(w_f32[:, :, D : 2 * D], wv_ext[:].rearrange("(j p) d -> p j d", p=128))
        nc.sync.dma_start(wq_f32[:], wq_ext[:].rearrange("(j p) d -> p j d", p=128))
        wkv = persist.tile([128, 4, 2 * D], BF16)
        wq = persist.tile([128, 4, D], BF16)
        nc.vector.tensor_copy(wkv[:], w_f32[:])
        nc.vector.tensor_copy(wq[:], wq_f32[:])

        bias_kv = persist.tile([128, 1], F32)
        nc.sync.dma_start(bias_kv[0:D, :], bk_ext[:].rearrange("(a b) -> a b", b=1))
        nc.sync.dma_start(bias_kv[D : 2 * D, :], bv_ext[:].rearrange("(a b) -> a b", b=1))
        bias_q = persist.tile([D, 1], F32)
        nc.sync.dma_start(bias_q[:], bq_ext[:].rearrange("(a b) -> a b", b=1))

        # ---------------- persistent per-batch tensors ----------------
        kvt = [persist.tile([128, S], BF16, name=f"kvt{b}", tag=f"kvt{b}") for b in range(B_LOC)]
        xtc = {}  # (b, c) -> rotating chunk tile [128, 4, 1024]
        qt_t = [persist.tile([128, Q_LOC], BF16, name=f"qt{b}", tag=f"qt{b}") for b in range(B_LOC)]
        vaug = [persist.tile([128, N_KT, D + 1], BF16, name=f"va{b}", tag=f"va{b}") for b in range(B_LOC)]

        def load_mask(kt, pool, tag):
            mk = pool.tile([128, Q_LOC], I32, name=f"mk{kt}", tag=tag)
            nc.sync.dma_start(mk[:], mt_ext[ts(kt, 128), :])
            return mk

        def emit_x_load(b: int, c: int, j: int):
            if (b, c) not in xtc:
                xtc[(b, c)] = xt_pool.tile(
                    [128, 4, 1024], BF16, name=f"xtc{b}_{c}", tag="xtc"
                )
            xs = xstage.tile([128, 1024], F32, tag="xs")
            nc.sync.dma_start(xs[:], xt_ext[b, ts(j, 128), ts(c, 1024)])
            if j % 2 == 0:
                nc.vector.tensor_copy(xtc[(b, c)][:, j, :], xs[:])
            else:
                nc.scalar.copy(xtc[(b, c)][:, j, :], xs[:])

        def emit_kv_half(b: int, c: int, h: int):
            # K^T|V^T for a 512-wide piece; one ps slot for ~2.5us only
            kv_ps = psum_s.tile([128, QC], F32, name="kvps", tag="ps")
            for j in range(4):
                nc.tensor.matmul(
                    kv_ps[:],
                    wkv[:, j, :],
                    xtc[(b, c)][:, j, ts(h, QC)],
                    start=(j == 0),
                    stop=(j == 3),
                )
            nc.scalar.activation(
                kvt[b][:, ds(c * 1024 + h * QC, QC)], kv_ps[:], AF.Identity,
                bias=bias_kv[:],
            )
            # V_aug rows for this half's 4 k-tiles via PE transpose
            vp = psum_s.tile([128, 4, D], BF16, name="vp", tag="ps")
            kt0 = 8 * c + 4 * h
            for u in range(4):
                nc.tensor.transpose(
                    vp[:, u, :],
                    kvt[b][D : 2 * D, ts(kt0 + u, 128)],
                    ident_b[D : 2 * D, D : 2 * D],
                )
            nc.scalar.copy(vaug[b][:, kt0 : kt0 + 4, 0:D], vp[:])

        def emit_q(b: int):
            q_ps = psum_s.tile([D, Q_LOC], F32, name="qps", tag="ps")
            for h in range(Q_LOC // QC):
                for j in range(4):
                    nc.tensor.matmul(
                        q_ps[:, ts(h, QC)],
                        wq[:, j, :],
                        xtc[(b, 0)][:, j, ts(h, QC)],
                        start=(j == 0),
                        stop=(j == 3),
                    )
            nc.scalar.activation(qt_t[b][0:D, :], q_ps[:], AF.Identity, bias=bias_q[:])
            nc.gpsimd.memset(qt_t[b][D:128, :], 0.0)
            nc.gpsimd.memset(vaug[b][:, :, D : D + 1], 1.0)

        def emit_scores_exp(b, kt, mk):
            st = psum_s.tile([128, Q_LOC], F32, name="st", tag="ps")
            for qc in range(Q_LOC // QC):
                nc.tensor.matmul(
                    st[:, ts(qc, QC)],
                    kvt[b][:, ts(kt, 128)],
                    qt_t[b][:, ts(qc, QC)],
                    start=True,
                    stop=True,
                )
            nc.vector.tensor_tensor(out=st[:], in0=st[:], in1=mk[:], op=ALU.mult)
            pt = pt_pool.tile([128, Q_LOC], BF16, tag="pt")
            nc.scalar.activation(pt[:], st[:], AF.Exp, scale=0.125)
            return pt

        def emit_pv(b, kt, ot, pt, first, last):
            for qc in range(Q_LOC // QC):
                nc.tensor.matmul(
                    ot[:, ts(qc, QC)],
                    vaug[b][:, kt, :],
                    pt[:, ts(qc, QC)],
                    start=first,
                    stop=last,
                )

        def emit_epilogue(b, ot):
            ots = epi.tile([D + 1, Q_LOC], F32, tag="ots")
            nc.scalar.copy(ots[:], ot[:])
            for qt in range(Q_LOC // 128):
                op = psum_s.tile([128, D + 1], F32, name="op", tag="ps")
                nc.tensor.transpose(
                    op[:], ots[:, ts(qt, 128)], ident_f[0 : D + 1, 0 : D + 1]
                )
                rcp = epi2.tile([128, 1], F32, tag="rcp")
                nc.vector.reciprocal(rcp[:], op[:, D : D + 1])
                of = epi2.tile([128, D], F32, tag="of")
                nc.vector.tensor_scalar(of[:], op[:, 0:D], rcp[:], None, op0=ALU.mult)
                nc.sync.dma_start(out_ext[b, ts(qt, 128), :], of[:])

        # ---------------- emission order (overlap hint) ----------------
        # PV matmuls lag one k-tile behind the scores so the in-order PE queue
        # never head-of-line blocks on the DVE/ACT chain (keeps PE dense/warm).
        ot0 = psum_o.tile([D + 1, Q_LOC], F32, name="ot0", tag="ot")
        ot1 = psum_o.tile([D + 1, Q_LOC], F32, name="ot1", tag="ot")
        ots = [ot0, ot1]
        N_C = 4
        for b in range(B_LOC):
            for j in range(4):
                emit_x_load(b, 0, j)
        for b in range(B_LOC):
            emit_kv_half(b, 0, 0)
            emit_kv_half(b, 0, 1)
            emit_q(b)
        pending = None  # (kt, [pt_b0, pt_b1])
        for c in range(N_C):
            nxt = []
            if c + 1 < N_C:
                nxt = (
                    [("x", b, c + 1, j) for b in range(B_LOC) for j in range(4)]
                    + [("kv", b, c + 1, h) for b in range(B_LOC) for h in range(2)]
                )
            for i, kt in enumerate(range(8 * c, 8 * c + 8)):
                mk = load_mask(kt, mstage, "mk")
                pts = [emit_scores_exp(b, kt, mk) for b in range(B_LOC)]
                if pending is not None:
                    pkt, ppts = pending
                    for b in range(B_LOC):
                        emit_pv(b, pkt, ots[b], ppts[b], pkt == 0, False)
                pending = (kt, pts)
                take = 2 if i < 4 else 1
                for _ in range(min(take, len(nxt))):
                    piece = nxt.pop(0)
                    if piece[0] == "x":
                        emit_x_load(*piece[1:])
                    else:
                        emit_kv_half(*piece[1:])
            for piece in nxt:
                if piece[0] == "x":
                    emit_x_load(*piece[1:])
                else:
                    emit_kv_half(*piece[1:])
        pkt, ppts = pending
        for b in range(B_LOC):
            emit_pv(b, pkt, ots[b], ppts[b], False, True)
        emit_epilogue(0, ot0)
        emit_epilogue(1, ot1)

    nc.compile()
    return nc


def _shard_inputs(input_embedding, mask, Wq, bq, Wk, bk, Wv, bv):
    input_embedding = np.asarray(input_embedding, dtype=np.float32)
    mask = np.asarray(mask, dtype=np.int32)
    w = {
        "wq": np.ascontiguousarray(np.asarray(Wq, np.float32)),
        "bq": np.ascontiguousarray(np.asarray(bq, np.float32)),
        "wk": np.ascontiguousarray(np.asarray(Wk, np.float32)),
        "bk": np.ascontiguousarray(np.asarray(bk, np.float32)),
        "wv": np.ascontiguousarray(np.asarray(Wv, np.float32)),
        "bv": np.ascontiguousarray(np.asarray(bv, np.float32)),
    }
    in_maps = []
    for c in range(N_CORES):
        bg, sq = divmod(c, 4)
        # x^T layout [2, C, S]; roll S so this core's q-slab is at [0:Q_LOC)
        x_c = np.roll(
            input_embedding[2 * bg : 2 * bg + 2].transpose(0, 2, 1),
            -Q_LOC * sq,
            axis=2,
        )
        # mask^T slab [S(k), Q_LOC(q)]; roll k-axis identically
        m_c = np.roll(mask[Q_LOC * sq : Q_LOC * (sq + 1), :].T, -Q_LOC * sq, axis=0)
        in_maps.append(
            {
                "xt": np.ascontiguousarray(x_c),
                "maskt": np.ascontiguousarray(m_c),
                **w,
            }
        )
    return in_maps


def _gather(results):
    out = np.empty((B, S, D), dtype=np.float32)
    for c in range(N_CORES):
        bg, sq = divmod(c, 4)
        out[2 * bg : 2 * bg + 2, Q_LOC * sq : Q_LOC * (sq + 1), :] = results[c]["out"]
    return out


def kernel(input_embedding, mask, Wq, bq, Wk, bk, Wv, bv):
    nc = build_kernel()
    in_maps = _shard_inputs(input_embedding, mask, Wq, bq, Wk, bk, Wv, bv)
    res = run_bass_kernel_spmd(nc, in_maps, list(range(N_CORES)))
    return _gather(res.results)


# revision 51
# speedup vs baseline: 1.2580x; 1.0430x over previous
"""Distributed masked-attention kernel for one TRN2 chip (8 NeuronCores).

Problem: B=4, S=4096, IN=512, D=64 attention with a [S,S] int32 score mask
(masked scores replaced by 1e-6 *before* softmax, so masked probs are
exp(1e-6)/Z ~= 1/Z, NOT zero).

Sharding (8 cores):
  core c = bg*4 + sq,  bg in {0,1} -> batches [2bg, 2bg+1],
  sq in {0..3} -> query rows [1024*sq, 1024*(sq+1)).
  Per-core inputs (layout chosen at scatter time):
    xt    = embedding[2bg:2bg+2].transpose(0,2,1)   [2, 512, 4096] f32
    maskt = mask[q_slab, :].T                       [4096, 1024]  int32
  Both are rolled along S so the core's own query slab is at rows [0:1024)
  (attention's k-sum is permutation invariant), letting all 8 cores run the
  IDENTICAL graph (SPMD).

Per-core device pipeline:
  QKV (per 1024-wide S-chunk, both batches interleaved so the DMA queue mixes
  x chunks with mask tiles): xt chunk cast f32->bf16 on DVE; K^T/V^T via
  packed [Wk|Wv] bf16 matmuls (biases fused into the ACT PSUM->SBUF copy);
  V^T chunk PE-transposed into V_aug=[V|1] tiles (ones column => the PV
  matmul emits the softmax denominator for free).
  Attention in the transposed domain S^T[k,q], both batches per k-tile so
  each streamed int32 mask tile is consumed twice and never stored:
    PE:  S^T = (K^T block)^T @ Q^T            (bf16, 2x N=512; Q^T is
         zero-padded to 128 partitions so the contraction uses the full PE
         array - required to un-throttle the PE HAM clock gate to 2.4GHz)
    DVE: sm = S^T * mask  (in-place PSUM)     (tensor_tensor, PSUM x int32)
    ACT: P = exp(0.125 * sm)                  (masked -> exp(0) = 1, matching
                                               the reference's exp(1e-6))
    PE:  O^T[65, q] += V_aug^T @ P            (2x N=512)
  The PV matmuls are emitted lagging one k-tile behind the scores so the
  in-order PE queue never head-of-line blocks on the DVE/ACT chain.
  Deep staging pools (xs x10, mask x16) keep >=10 DMA transfers in flight:
  each HWDGE dma_start runs on ONE queue at ~27GB/s, so aggregate HBM
  bandwidth equals 27GB/s x outstanding transfers.
  Epilogue: PE-transpose O^T, divide by the denominator row, DMA out.
"""

import sys

if "/opt/trn_rl_repo" not in sys.path:
    sys.path.insert(0, "/opt/trn_rl_repo")

from contextlib import ExitStack

import numpy as np

import concourse.bass as bass
import concourse.bacc as bacc
import concourse.mybir as mybir
import concourse.tile as tile
from concourse.bass_utils import run_bass_kernel_spmd
from concourse.masks import make_identity

ts = bass.ts
ds = bass.ds

N_CORES = 8
B, S, C, D = 4, 4096, 512, 64
B_LOC = 2          # batches per core
Q_LOC = 1024       # query rows per core
N_KT = S // 128    # 32 k-tiles of 128
QC = 512           # matmul moving chunk

F32 = mybir.dt.float32
F32R = mybir.dt.float32r
BF16 = mybir.dt.bfloat16
I32 = mybir.dt.int32
AF = mybir.ActivationFunctionType
ALU = mybir.AluOpType


def build_kernel() -> bacc.Bacc:
    nc = bacc.Bacc(None, target_bir_lowering=False, debug=False)

    xt_ext = nc.declare_dram_parameter("xt", [B_LOC, C, S], F32, isOutput=False)
    mt_ext = nc.declare_dram_parameter("maskt", [S, Q_LOC], I32, isOutput=False)
    wq_ext = nc.declare_dram_parameter("wq", [C, D], F32, isOutput=False)
    bq_ext = nc.declare_dram_parameter("bq", [D], F32, isOutput=False)
    wk_ext = nc.declare_dram_parameter("wk", [C, D], F32, isOutput=False)
    bk_ext = nc.declare_dram_parameter("bk", [D], F32, isOutput=False)
    wv_ext = nc.declare_dram_parameter("wv", [C, D], F32, isOutput=False)
    bv_ext = nc.declare_dram_parameter("bv", [D], F32, isOutput=False)
    out_ext = nc.declare_dram_parameter("out", [B_LOC, Q_LOC, D], F32, isOutput=True)

    with tile.TileContext(nc) as tc, ExitStack() as ctx:
        # ---------------- pools ----------------
        persist = ctx.enter_context(tc.tile_pool(name="persist", bufs=1))
        xt_pool = ctx.enter_context(tc.tile_pool(name="xtp", bufs=3))
        mstage = ctx.enter_context(tc.tile_pool(name="mstage", bufs=16))
        xstage = ctx.enter_context(tc.tile_pool(name="xstage", bufs=10))
        pt_pool = ctx.enter_context(tc.tile_pool(name="pt", bufs=6))
        epi = ctx.enter_context(tc.tile_pool(name="epi", bufs=1))
        epi2 = ctx.enter_context(tc.tile_pool(name="epi2", bufs=2))
        psum_s = ctx.enter_context(
            tc.tile_pool(name="psum_s", bufs=2, space=bass.MemorySpace.PSUM)
        )
        psum_o = ctx.enter_context(
            tc.tile_pool(name="psum_o", bufs=2, space=bass.MemorySpace.PSUM)
        )

        # ---------------- constants / weights ----------------
        ident_f = persist.tile([128, 128], F32)
        make_identity(nc, ident_f[:])
        ident_b = persist.tile([128, 128], BF16)
        make_identity(nc, ident_b[:])
        ones_col = persist.tile([128, 1], BF16)
        nc.gpsimd.memset(ones_col[:], 1.0)

        # [Wk | Wv] packed bf16 stationary blocks; Wq separate
        w_f32 = persist.tile([128, 4, 2 * D], F32)
        wq_f32 = persist.tile([128, 4, D], F32)
        nc.sync.dma_start(w_f32[:, :, 0:D], wk_ext[:].rearrange("(j p) d -> p j d", p=128))
        nc.sync.dma_start(w_f32[:, :, D : 2 * D], wv_ext[:].rearrange("(j p) d -> p j d", p=128))
        nc.sync.dma_start(wq_f32[:], wq_ext[:].rearrange("(j p) d -> p j d", p=128))
        wkv = persist.tile([128, 4, 2 * D], BF16)
        wq = persist.tile([128, 4, D], BF16)
        nc.vector.tensor_copy(wkv[:], w_f32[:])
        nc.vector.tensor_copy(wq[:], wq_f32[:])

        bias_kv = persist.tile([128, 1], F32)
        nc.sync.dma_start(bias_kv[0:D, :], bk_ext[:].rearrange("(a b) -> a b", b=1))
        nc.sync.dma_start(bias_kv[D : 2 * D, :], bv_ext[:].rearrange("(a b) -> a b", b=1))
        bias_q = persist.tile([D, 1], F32)
        nc.sync.dma_start(bias_q[:], bq_ext[:].rearrange("(a b) -> a b", b=1))

        # ---------------- persistent per-batch tensors ----------------
        kvt = [persist.tile([128, S], BF16, name=f"kvt{b}", tag=f"kvt{b}") for b in range(B_LOC)]
        xtc = {}  # (b, c) -> rotating chunk tile [128, 4, 1024]
        qt_t = [persist.tile([128, Q_LOC], BF16, name=f"qt{b}", tag=f"qt{b}") for b in range(B_LOC)]
        vaug = [persist.tile([128, N_KT, D + 1], BF16, name=f"va{b}", tag=f"va{b}") for b in range(B_LOC)]

        def load_mask(kt, pool, tag):
            mk = pool.tile([128, Q_LOC], I32, name=f"mk{kt}", tag=tag)
            nc.sync.dma_start(mk[:], mt_ext[ts(kt, 128), :])
            return mk

        def emit_x_load(b: int, c: int, j: int):
            if (b, c) not in xtc:
                xtc[(b, c)] = xt_pool.tile(
                    [128, 4, 1024], BF16, name=f"xtc{b}_{c}", tag="xtc"
                )
            xs = xstage.tile([128, 1024], F32, tag="xs")
            nc.sync.dma_start(xs[:], xt_ext[b, ts(j, 128), ts(c, 1024)])
            if j % 2 == 0:
                nc.vector.tensor_copy(xtc[(b, c)][:, j, :], xs[:])
            else:
                nc.scalar.copy(xtc[(b, c)][:, j, :], xs[:])

        def emit_kv_half(b: int, c: int, h: int):
            # K^T|V^T for a 512-wide piece; one ps slot for ~2.5us only
            kv_ps = psum_s.tile([128, QC], F32, name="kvps", tag="ps")
            for j in range(4):
                nc.tensor.matmul(
                    kv_ps[:],
                    wkv[:, j, :],
                    xtc[(b, c)][:, j, ts(h, QC)],
                    start=(j == 0),
                    stop=(j == 3),
                )
            nc.scalar.activation(
                kvt[b][:, ds(c * 1024 + h * QC, QC)], kv_ps[:], AF.Identity,
                bias=bias_kv[:],
            )
            # V_aug rows for this half's 4 k-tiles via PE transpose
            vp = psum_s.tile([128, 4, D], BF16, name="vp", tag="ps")
            kt0 = 8 * c + 4 * h
            for u in range(4):
                nc.tensor.transpose(
                    vp[:, u, :],
                    kvt[b][D : 2 * D, ts(kt0 + u, 128)],
                    ident_b[D : 2 * D, D : 2 * D],
                )
            nc.scalar.copy(vaug[b][:, kt0 : kt0 + 4, 0:D], vp[:])

        def emit_q(b: int):
            q_ps = psum_s.tile([D, Q_LOC], F32, name="qps", tag="ps")
            for h in range(Q_LOC // QC):
                for j in range(4):
                    nc.tensor.matmul(
                        q_ps[:, ts(h, QC)],
                        wq[:, j, :],
                        xtc[(b, 0)][:, j, ts(h, QC)],
                        start=(j == 0),
                        stop=(j == 3),
                    )
            nc.scalar.activation(qt_t[b][0:D, :], q_ps[:], AF.Identity, bias=bias_q[:])
            nc.gpsimd.memset(qt_t[b][D:128, :], 0.0)
            nc.gpsimd.memset(vaug[b][:, :, D : D + 1], 1.0)

        def emit_scores_exp(b, kt, mk):
            st = psum_s.tile([128, Q_LOC], F32, name="st", tag="ps")
            for qc in range(Q_LOC // QC):
                nc.tensor.matmul(
                    st[:, ts(qc, QC)],
                    kvt[b][:, ts(kt, 128)],
                    qt_t[b][:, ts(qc, QC)],
                    start=True,
                    stop=True,
                )
            nc.vector.tensor_tensor(out=st[:], in0=st[:], in1=mk[:], op=ALU.mult)
            pt = pt_pool.tile([128, Q_LOC], BF16, tag="pt")
            nc.scalar.activation(pt[:], st[:], AF.Exp, scale=0.125)
            return pt

        def emit_pv(b, kt, ot, pt, first, last):
            for qc in range(Q_LOC // QC):
                nc.tensor.matmul(
                    ot[:, ts(qc, QC)],
                    vaug[b][:, kt, :],
                    pt[:, ts(qc, QC)],
                    start=first,
                    stop=last,
                )

        def emit_epilogue(b, ot):
            ots = epi.tile([D + 1, Q_LOC], F32, tag="ots")
            nc.scalar.copy(ots[:], ot[:])
            # batch all 8 q-block transposes into one 2-bank PSUM tile,
            # divide, and store with a single DMA
            op8 = psum_s.tile([128, 8, 128], F32, name="op8", tag="ps")
            for qt in range(Q_LOC // 128):
                nc.tensor.transpose(
                    op8[:, qt, 0 : D + 1], ots[:, ts(qt, 128)],
                    ident_f[0 : D + 1, 0 : D + 1],
                )
            rcp = epi2.tile([128, 8], F32, tag="rcp")
            for qt in range(Q_LOC // 128):
                nc.vector.reciprocal(rcp[:, qt : qt + 1], op8[:, qt, D : D + 1])
            of = epi2.tile([128, 8, D], F32, tag="of")
            for qt in range(Q_LOC // 128):
                nc.vector.tensor_scalar(
                    of[:, qt, :], op8[:, qt, 0:D], rcp[:, qt : qt + 1], None,
                    op0=ALU.mult,
                )
            nc.sync.dma_start(
                out_ext[b].rearrange("(qt p) d -> p qt d", p=128), of[:]
            )

        # ---------------- emission order (overlap hint) ----------------
        # PV matmuls lag one k-tile behind the scores so the in-order PE queue
        # never head-of-line blocks on the DVE/ACT chain (keeps PE dense/warm).
        ot0 = psum_o.tile([D + 1, Q_LOC], F32, name="ot0", tag="ot")
        ot1 = psum_o.tile([D + 1, Q_LOC], F32, name="ot1", tag="ot")
        ots = [ot0, ot1]
        N_C = 4
        for b in range(B_LOC):
            for j in range(4):
                emit_x_load(b, 0, j)
        for b in range(B_LOC):
            emit_kv_half(b, 0, 0)
            emit_kv_half(b, 0, 1)
            emit_q(b)
        pending = None  # (kt, [pt_b0, pt_b1])
        for c in range(N_C):
            nxt = []
            if c + 1 < N_C:
                nxt = (
                    [("x", b, c + 1, j) for b in range(B_LOC) for j in range(4)]
                    + [("kv", b, c + 1, h) for b in range(B_LOC) for h in range(2)]
                )
            for i, kt in enumerate(range(8 * c, 8 * c + 8)):
                mk = load_mask(kt, mstage, "mk")
                pts = [emit_scores_exp(b, kt, mk) for b in range(B_LOC)]
                if pending is not None:
                    pkt, ppts = pending
                    for b in range(B_LOC):
                        emit_pv(b, pkt, ots[b], ppts[b], pkt == 0, False)
                pending = (kt, pts)
                take = 2 if i < 4 else 1
                for _ in range(min(take, len(nxt))):
                    piece = nxt.pop(0)
                    if piece[0] == "x":
                        emit_x_load(*piece[1:])
                    else:
                        emit_kv_half(*piece[1:])
            for piece in nxt:
                if piece[0] == "x":
                    emit_x_load(*piece[1:])
                else:
                    emit_kv_half(*piece[1:])
        pkt, ppts = pending
        for b in range(B_LOC):
            emit_pv(b, pkt, ots[b], ppts[b], False, True)
        emit_epilogue(0, ot0)
        emit_epilogue(1, ot1)

    nc.compile()
    return nc


def _shard_inputs(input_embedding, mask, Wq, bq, Wk, bk, Wv, bv):
    input_embedding = np.asarray(input_embedding, dtype=np.float32)
    mask = np.asarray(mask, dtype=np.int32)
    w = {
        "wq": np.ascontiguousarray(np.asarray(Wq, np.float32)),
        "bq": np.ascontiguousarray(np.asarray(bq, np.float32)),
        "wk": np.ascontiguousarray(np.asarray(Wk, np.float32)),
        "bk": np.ascontiguousarray(np.asarray(bk, np.float32)),
        "wv": np.ascontiguousarray(np.asarray(Wv, np.float32)),
        "bv": np.ascontiguousarray(np.asarray(bv, np.float32)),
    }
    in_maps = []
    for c in range(N_CORES):
        bg, sq = divmod(c, 4)
        # x^T layout [2, C, S]; roll S so this core's q-slab is at [0:Q_LOC)
        x_c = np.roll(
            input_embedding[2 * bg : 2 * bg + 2].transpose(0, 2, 1),
            -Q_LOC * sq,
            axis=2,
        )
        # mask^T slab [S(k), Q_LOC(q)]; roll k-axis identically
        m_c = np.roll(mask[Q_LOC * sq : Q_LOC * (sq + 1), :].T, -Q_LOC * sq, axis=0)
        in_maps.append(
            {
                "xt": np.ascontiguousarray(x_c),
                "maskt": np.ascontiguousarray(m_c),
                **w,
            }
        )
    return in_maps


def _gather(results):
    out = np.empty((B, S, D), dtype=np.float32)
    for c in range(N_CORES):
        bg, sq = divmod(c, 4)
        out[2 * bg : 2 * bg + 2, Q_LOC * sq : Q_LOC * (sq + 1), :] = results[c]["out"]
    return out


def kernel(input_embedding, mask, Wq, bq, Wk, bk, Wv, bv):
    nc = build_kernel()
    in_maps = _shard_inputs(input_embedding, mask, Wq, bq, Wk, bk, Wv, bv)
    res = run_bass_kernel_spmd(nc, in_maps, list(range(N_CORES)))
    return _gather(res.results)
